# revision 1
# baseline (speedup 1.0000x reference)
"""DMPNet Trainium2 kernel.

Strategy
--------
* Pure batch data parallelism: 16384 rows -> 8 cores x 2048.
* The MLP (128 -> 2048 -> 2048 -> 54, tanh) runs on the tensor engine in
  float32r (fp32 with 11 explicit mantissa bits): full PE rate for moving
  dim >= 256 and fp32-exact accumulation.  Inputs/weights are pre-rounded
  to fp32r on the host.
* The 101-step DMP Euler integration is a linear time-invariant recurrence
  in (y, z); it collapses exactly into
      out[r, j] = da_j*y0 + db_j*dy0 + dg_j*goal + (goal - y0) * (w @ dQ_j)
  with coefficients precomputed on the host in float64.  The (w @ dQ) part
  is folded into the final-layer weights (W_eff), so the device only runs
  3 matmul layers + 2 tiny broadcast matmuls + 2 elementwise ops.
* All activations live feature-major ([feature, batch]) so no transposes
  are needed on device; the input is transposed host-side.
"""

import os

import numpy as np

import concourse.bass as bass
import concourse.mybir as mybir
from concourse import bacc
from concourse.tile import TileContext
from concourse.bass_utils import run_bass_kernel_spmd

F32 = mybir.dt.float32
F32R = mybir.dt.float32r

N_CORES = 8
B_TOTAL = 16384
B_SH = B_TOTAL // N_CORES          # 2048 rows per core
D_IN = 128
H = 2048
HC = H // 128                      # 16 chunks of 128
DIM = 9
N_BASIS = 5
NOUT = 10                          # output time steps
M_S = DIM * NOUT                   # 90 "S" rows
M_ALL = M_S + DIM                  # 99 rows of the effective final layer

TW = int(os.environ.get("DMP_TW", "512"))            # batch tile width
REPEAT = int(os.environ.get("DMP_KERNEL_REPEAT", "1"))
NT = B_SH // TW

_TANH = mybir.ActivationFunctionType.Tanh
_IDENT = mybir.ActivationFunctionType.Identity


def _round_fp32r(x: np.ndarray) -> np.ndarray:
    """Round fp32 -> fp32r (11 explicit mantissa bits), nearest-even."""
    b = np.ascontiguousarray(x, dtype=np.float32).view(np.uint32)
    lsb = (b >> np.uint32(12)) & np.uint32(1)
    r = b + (np.uint32(0x7FF) + lsb)
    r &= np.uint32(0xFFFFF000)
    return r.view(np.float32)


def _dmp_coefficients():
    """Closed-form coefficients of the sampled-position differences.

    Returns (d_alpha, d_beta, d_gamma, dQ) with dQ shaped (NOUT, N_BASIS):
      out[r, j] = d_alpha[j]*y0 + d_beta[j]*dy0 + d_gamma[j]*goal
                  + (goal - y0) * sum_n w[r, n] * dQ[j, n]
    """
    A_X, A_Z, TAU, DT = 1.0, 25.0, 1.0, 0.01
    B_Z = A_Z / 4.0
    NSTEP, L_SUB = 100, 10

    c = np.exp(-A_X * np.linspace(0.0, 1.0, N_BASIS))
    h = N_BASIS ** 1.5 / c / A_X
    xs = (1.0 - A_X * DT / TAU) ** np.arange(1, NSTEP + 1)
    psi = np.exp(-h[None, :] * (xs[:, None] - c[None, :]) ** 2)
    p = psi * xs[:, None] / psi.sum(axis=1, keepdims=True)      # (100, 5)

    nb = 3 + NSTEP
    cy = np.zeros(nb)
    cz = np.zeros(nb)
    cy[0] = 1.0
    cz[1] = TAU
    ys = [cy.copy()]
    for k in range(NSTEP):
        dz = np.zeros(nb)
        dz[2] = A_Z * B_Z
        dz -= A_Z * B_Z * cy
        dz -= A_Z * cz
        dz[3 + k] += 1.0
        dz /= TAU
        dy = cz / TAU
        cy = cy + dy * DT
        cz = cz + dz * DT
        ys.append(cy.copy())
    ys = np.array(ys)                         # (101, 103)
    samp = ys[::L_SUB]                        # (11, 103)
    dcoef = samp[1:] - samp[:-1]              # (10, 103)
    dQ = dcoef[:, 3:] @ p                     # (10, 5)
    return dcoef[:, 0], dcoef[:, 1], dcoef[:, 2], dQ


_NC_CACHE = {}


def _build_program(tw: int, repeat: int):
    nt = B_SH // tw
    nc = bacc.Bacc()

    xT = nc.dram_tensor("xT", [D_IN, B_SH], F32R, kind="ExternalInput")
    w0t = nc.dram_tensor("w0t", [D_IN, H], F32R, kind="ExternalInput")
    b0d = nc.dram_tensor("b0d", [128, HC], F32, kind="ExternalInput")
    w1t = nc.dram_tensor("w1t", [H, H], F32R, kind="ExternalInput")
    b1d = nc.dram_tensor("b1d", [128, HC], F32, kind="ExternalInput")
    weff = nc.dram_tensor("weff", [H, M_ALL], F32R, kind="ExternalInput")
    beff = nc.dram_tensor("beff", [M_ALL, 1], F32, kind="ExternalInput")
    linc = nc.dram_tensor("linc", [27, M_S], F32R, kind="ExternalInput")
    diffc = nc.dram_tensor("diffc", [27, M_S], F32R, kind="ExternalInput")
    outT = nc.dram_tensor("outT", [M_S, B_SH], F32, kind="ExternalOutput")

    with TileContext(nc) as tc:
        with (
            tc.tile_pool(name="wres", bufs=1) as wres,
            tc.tile_pool(name="io", bufs=1) as io,
            tc.tile_pool(name="h0p", bufs=16) as h0p,
            tc.tile_pool(name="h1p", bufs=2) as h1p,
            tc.tile_pool(name="outp", bufs=1) as outp,
            tc.tile_pool(name="ps_scr", bufs=2, space="PSUM") as ps_scr,
            tc.tile_pool(name="ps_h1", bufs=2, space="PSUM") as ps_h1,
            tc.tile_pool(name="ps_m", bufs=2, space="PSUM") as ps_m,
        ):
            # ---- resident weights ----
            w1_sb = wres.tile([128, HC, H], F32R, tag="w1")
            for i in range(HC):
                nc.sync.dma_start(out=w1_sb[:, i, :], in_=w1t[i * 128:(i + 1) * 128, :])
            w0_sb = wres.tile([128, H], F32R, tag="w0")
            nc.sync.dma_start(out=w0_sb, in_=w0t[:, :])
            weff_sb = wres.tile([128, HC, M_ALL], F32R, tag="weff")
            for i in range(HC):
                nc.sync.dma_start(out=weff_sb[:, i, :], in_=weff[i * 128:(i + 1) * 128, :])
            b0_sb = wres.tile([128, HC], F32, tag="b0")
            nc.sync.dma_start(out=b0_sb, in_=b0d[:, :])
            b1_sb = wres.tile([128, HC], F32, tag="b1")
            nc.sync.dma_start(out=b1_sb, in_=b1d[:, :])
            beff_sb = wres.tile([M_ALL, 1], F32, tag="beff")
            nc.sync.dma_start(out=beff_sb, in_=beff[:, :])
            linc_sb = wres.tile([27, M_S], F32R, tag="linc")
            nc.sync.dma_start(out=linc_sb, in_=linc[:, :])
            diffc_sb = wres.tile([27, M_S], F32R, tag="diffc")
            nc.sync.dma_start(out=diffc_sb, in_=diffc[:, :])

            for _rep in range(repeat):
                for t in range(nt):
                    win = slice(t * tw, (t + 1) * tw)

                    x_sb = io.tile([128, tw], F32R, tag="x")
                    nc.sync.dma_start(out=x_sb, in_=xT[:, win])
                    stacked = io.tile([27, tw], F32R, tag="stk")
                    nc.sync.dma_start(out=stacked[0:9, :], in_=xT[7:16, win])
                    nc.sync.dma_start(out=stacked[9:18, :], in_=xT[22:31, win])

                    # ---- layer 0 ----
                    h0_tiles = []
                    for c in range(HC):
                        ps = ps_scr.tile([128, tw], F32, tag="scr")
                        nc.tensor.matmul(
                            ps, w0_sb[:, c * 128:(c + 1) * 128], x_sb,
                            start=True, stop=True,
                        )
                        h0c = h0p.tile([128, tw], F32R, tag="h0")
                        nc.scalar.activation(
                            out=h0c, in_=ps, func=_TANH, bias=b0_sb[:, c:c + 1],
                        )
                        h0_tiles.append(h0c)

                    # ---- layer 1 + interleaved final-layer accumulation ----
                    psm = ps_m.tile([M_ALL, tw], F32, tag="m")
                    for j in range(HC):
                        ps1 = ps_h1.tile([128, tw], F32, tag="h1")
                        for i in range(HC):
                            nc.tensor.matmul(
                                ps1, w1_sb[:, i, j * 128:(j + 1) * 128], h0_tiles[i],
                                start=(i == 0), stop=(i == HC - 1),
                            )
                        h1c = h1p.tile([128, tw], F32R, tag="h1c")
                        nc.scalar.activation(
                            out=h1c, in_=ps1, func=_TANH, bias=b1_sb[:, j:j + 1],
                        )
                        nc.tensor.matmul(
                            psm, weff_sb[:, j, :], h1c,
                            start=(j == 0), stop=(j == HC - 1),
                            skip_group_check=True,
                        )

                    mlp2 = io.tile([M_ALL, tw], F32R, tag="m2")
                    nc.scalar.activation(
                        out=mlp2, in_=psm, func=_IDENT, bias=beff_sb[:, 0:1],
                    )
                    nc.sync.dma_start(out=stacked[18:27, :], in_=mlp2[M_S:M_ALL, :])

                    lin_ps = ps_scr.tile([M_S, tw], F32, tag="scr")
                    nc.tensor.matmul(lin_ps, linc_sb, stacked, start=True, stop=True)
                    diff_ps = ps_scr.tile([M_S, tw], F32, tag="scr")
                    nc.tensor.matmul(diff_ps, diffc_sb, stacked, start=True, stop=True)

                    prod = outp.tile([M_S, tw], F32, tag="prod")
                    nc.vector.tensor_mul(prod, diff_ps, mlp2[0:M_S, :].bitcast(F32))
                    res = outp.tile([M_S, tw], F32, tag="res")
                    nc.vector.tensor_add(res, prod, lin_ps)
                    nc.sync.dma_start(out=outT[:, win], in_=res)

    nc.compile()
    return nc


def _get_program(tw: int = TW, repeat: int = REPEAT):
    key = (tw, repeat)
    if key not in _NC_CACHE:
        _NC_CACHE[key] = _build_program(tw, repeat)
    return _NC_CACHE[key]


def _prepare_host_inputs(input, W0, b0, W1, b1, Wl, bl):
    """Build the per-core input maps (host-side prep, float64 coefficients)."""
    d_alpha, d_beta, d_gamma, dQ = _dmp_coefficients()

    Wl100 = Wl.astype(np.float64) * 100.0          # (54, H)
    bl100 = bl.astype(np.float64) * 100.0          # (54,)

    # effective final layer: rows 0..89 = S rows (d*10+j), 90..98 = goal rows
    weff = np.zeros((H, M_ALL), dtype=np.float64)
    beff = np.zeros((M_ALL,), dtype=np.float64)
    for d in range(DIM):
        for j in range(NOUT):
            m = d * NOUT + j
            wrow = np.zeros(H, dtype=np.float64)
            brow = 0.0
            for n in range(N_BASIS):
                wrow += dQ[j, n] * Wl100[DIM + N_BASIS * d + n]
                brow += dQ[j, n] * bl100[DIM + N_BASIS * d + n]
            weff[:, m] = wrow
            beff[m] = brow
        weff[:, M_S + d] = Wl100[d]
        beff[M_S + d] = bl100[d]

    # broadcast matmul constants [27, 90]
    linc = np.zeros((27, M_S), dtype=np.float64)
    diffc = np.zeros((27, M_S), dtype=np.float64)
    for d in range(DIM):
        for j in range(NOUT):
            m = d * NOUT + j
            linc[d, m] = d_alpha[j]
            linc[9 + d, m] = d_beta[j]
            linc[18 + d, m] = d_gamma[j]
            diffc[d, m] = -1.0
            diffc[18 + d, m] = 1.0

    shared = {
        "w0t": _round_fp32r(np.ascontiguousarray(W0.T)),
        "b0d": np.ascontiguousarray(np.asarray(b0, np.float32).reshape(HC, 128).T),
        "w1t": _round_fp32r(np.ascontiguousarray(W1.T)),
        "b1d": np.ascontiguousarray(np.asarray(b1, np.float32).reshape(HC, 128).T),
        "weff": _round_fp32r(weff),
        "beff": np.ascontiguousarray(beff.astype(np.float32).reshape(M_ALL, 1)),
        "linc": _round_fp32r(linc),
        "diffc": _round_fp32r(diffc),
    }

    xr = _round_fp32r(np.asarray(input, np.float32))
    in_maps = []
    for c in range(N_CORES):
        m = dict(shared)
        m["xT"] = np.ascontiguousarray(xr[c * B_SH:(c + 1) * B_SH, :].T)
        in_maps.append(m)
    return in_maps


def kernel(input, W0, b0, W1, b1, Wl, bl):
    nc = _get_program()
    in_maps = _prepare_host_inputs(input, W0, b0, W1, b1, Wl, bl)
    results = run_bass_kernel_spmd(nc, in_maps, core_ids=list(range(N_CORES)))
    outs = []
    for c in range(N_CORES):
        o = results.results[c]["outT"]                     # (90, 2048)
        outs.append(o.reshape(DIM, NOUT, B_SH).transpose(2, 0, 1))
    return np.ascontiguousarray(np.concatenate(outs, axis=0), dtype=np.float32)
